# revision 18
# baseline (speedup 1.0000x reference)
"""BlockAttention TRN2 Bass kernel (bf16, fused local/cross attend pairs).

Problem (hardcoded): x [4, 4096, 1024] fp32; wq/wk/wv/wo [1024, 1024];
bq/bk/bv/bo [1024]; block_size 256. Output [4, 8192, 1024]:
per 256-token block g: rows [512g, 512g+256) = softmax(Q_g K_g^T / 32) V_g @ wo,
rows [512g+256, 512g+512) = softmax(Q_g K_{g-1}^T / 32) V_{g-1} @ wo (block 0
attends to itself), all + bo.

Sharding: 8 cores = 4 batches x 2 sequence halves. Each core gets x^T (bf16)
for 9 kv blocks (prev + its 8; even cores duplicate block 0 as "prev"), all
weights (bf16), and writes out^T [1024, 4096] bf16 for its 4096 output rows.

All matmul operands are bf16 (same PE stream rate as fp32r, but FWL halves
weight-load time, SBUF/DMA traffic halves). N=512 free dims throughout:
  - K/V/Q projections computed two blocks at a time from one resident x tile.
  - VW = V @ wo per kv block (halves out-proj flops; out = P @ VW).
  - Attends are fused PAIRS keyed by kv block k: the local softmax of q-block
    k and the cross softmax of q-block k+1 both attend keys(k), so their
    score/out matmuls share stationary operands and run at N=512:
      S^T[keys(k), q(k)|q(k+1)] -> exp -> key-sums (ones matmul broadcast)
      -> reciprocal_approx_fast -> out^T = VW(k)^T P, normalized on DVE.
bo is added on the host (exact, zero-cost on device).
"""

import numpy as np
import ml_dtypes
from contextlib import ExitStack

import concourse.bass as bass
import concourse.mybir as mybir
import concourse.tile as tile
from concourse import bacc, bass_utils

D = 1024
BS = 256
NKV = 9  # kv blocks per core (prev + 8 own)
TKV = NKV * BS  # 2304
NQT = 8 * BS  # 2048 q tokens (blocks 1..8)
DS = D // 128  # 8 subtiles of the feature dim
F32 = mybir.dt.float32
BF = mybir.dt.bfloat16
SCALE = 1.0 / 32.0  # 1/sqrt(D)
BF_NP = ml_dtypes.bfloat16

_CACHED_NC = None


def _build():
    nc = bacc.Bacc("TRN2", target_bir_lowering=False, debug=False, num_devices=8)
    xt = nc.dram_tensor("xt", [D, TKV], BF, kind="ExternalInput").ap()
    w_ap = {
        n: nc.dram_tensor(n, [D, D], BF, kind="ExternalInput").ap()
        for n in ("wq", "wk", "wv", "wo")
    }
    b_ap = {
        n: nc.dram_tensor(n, [128, DS], F32, kind="ExternalInput").ap()
        for n in ("bq", "bk", "bv")
    }
    ones2d = nc.dram_tensor("ones2d", [128, 128], BF, kind="ExternalInput").ap()
    outt = nc.dram_tensor("outt", [D, 8 * 2 * BS], BF, kind="ExternalOutput").ap()

    Ident = mybir.ActivationFunctionType.Identity
    Exp = mybir.ActivationFunctionType.Exp

    with (
        tile.TileContext(nc) as tc,
        ExitStack() as ctx,
        nc.allow_low_precision(reason="bf16 matmul operands by design"),
    ):
        wp = ctx.enter_context(tc.tile_pool(name="wp", bufs=1))
        cp = ctx.enter_context(tc.tile_pool(name="cp", bufs=1))
        xp = ctx.enter_context(tc.tile_pool(name="xp", bufs=1))
        qp = ctx.enter_context(tc.tile_pool(name="qp", bufs=1))
        kp = ctx.enter_context(tc.tile_pool(name="kp", bufs=3))
        vp = ctx.enter_context(tc.tile_pool(name="vp", bufs=2))
        wvp = ctx.enter_context(tc.tile_pool(name="wvp", bufs=3))
        pp = ctx.enter_context(tc.tile_pool(name="pp", bufs=2))
        rp = ctx.enter_context(tc.tile_pool(name="rp", bufs=2))
        op_sb = ctx.enter_context(tc.tile_pool(name="op_sb", bufs=10))
        PSUM = bass.MemorySpace.PSUM
        ps_pr = ctx.enter_context(tc.tile_pool(name="ps_pr", bufs=2, space=PSUM))
        ps_sc = ctx.enter_context(tc.tile_pool(name="ps_sc", bufs=2, space=PSUM))
        ps_op = ctx.enter_context(tc.tile_pool(name="ps_op", bufs=3, space=PSUM))
        ps_bc = ctx.enter_context(tc.tile_pool(name="ps_bc", bufs=1, space=PSUM))

        # Resident big tiles. x^T: all 9 kv blocks; Q^T: blocks 1..8 written
        # by the Q projections (contiguous token windows let score matmuls
        # stream [Q(k)|Q(k+1)] at N=512 across q-block boundaries).
        x_sb = xp.tile([128, DS, TKV], BF, tag="x")
        qt = qp.tile([128, DS, NQT], BF, tag="qt")

        w_sb = {}

        def load_w(n, engs=None):
            t = wp.tile([128, DS, D], BF, tag=n)
            engs = engs or (nc.sync, nc.scalar)
            for s in range(DS):
                eng = engs[s % len(engs)]
                eng.dma_start(t[:, s, :], w_ap[n][128 * s : 128 * (s + 1), :])
            w_sb[n] = t

        b_sb = {}
        for n in ("bq", "bk", "bv"):
            t = cp.tile([128, DS], F32, tag=n)
            nc.scalar.dma_start(t[:], b_ap[n])
            b_sb[n] = t
        ones_sb = cp.tile([128, 128], BF, tag="ones")
        nc.scalar.dma_start(ones_sb[:], ones2d)

        # PE warmup: the HAM clock gate leaves the PE at 1.2 GHz until it has
        # seen ~3.4us of sustained activity, and the DMA-bound head would
        # otherwise keep it cold (and bursty) until ~30us in. Chew on the
        # ones tile until the real first matmul is ready.
        for wi in range(112):
            ps_wu = ps_sc.tile([128, 512], F32, tag="sc")
            nc.tensor.matmul(ps_wu[:, :128], ones_sb[:], ones_sb[:])

        def load_x(chunks):
            # x arrives in 512-token column chunks so early projection groups
            # unblock before the whole tensor lands.
            for c in chunks:
                c0, c1 = 512 * c, min(512 * (c + 1), TKV)
                for s in range(DS):
                    nc.gpsimd.dma_start(
                        x_sb[:, s, c0:c1], xt[128 * s : 128 * (s + 1), c0:c1]
                    )

        def proj(wname, tok0, ntok):
            # (W^T x^T)[d_out, tok] per m-subtile into one PSUM bank.
            for m in range(DS):
                pst = ps_pr.tile([128, 512], F32, tag="pr")
                for kk in range(DS):
                    nc.tensor.matmul(
                        pst[:, :ntok],
                        w_sb[wname][:, kk, 128 * m : 128 * (m + 1)],
                        x_sb[:, kk, tok0 : tok0 + ntok],
                        start=(kk == 0),
                        stop=(kk == DS - 1),
                    )
                yield m, pst

        def proj_k(blocks):
            # K^T for 1 or 2 blocks -> per-block tiles [128, DS, BS]
            kts = [
                kp.tile([128, DS, BS], BF, tag="kt", name=f"kt{i}")
                for i in range(len(blocks))
            ]
            for m, pst in proj("wk", BS * blocks[0], BS * len(blocks)):
                for i in range(len(blocks)):
                    nc.scalar.activation(
                        kts[i][:, m, :],
                        pst[:, BS * i : BS * (i + 1)],
                        Ident,
                        bias=b_sb["bk"][:, m : m + 1],
                    )
            return kts

        def proj_q(g0, ng):
            # Q^T for q-blocks [g0, g0+ng) into the resident qt tile.
            c0 = BS * (g0 - 1)
            for m, pst in proj("wq", BS * g0, BS * ng):
                nc.scalar.activation(
                    qt[:, m, c0 : c0 + BS * ng],
                    pst[:, : BS * ng],
                    Ident,
                    bias=b_sb["bq"][:, m : m + 1],
                )

        def proj_v(blocks):
            vt = vp.tile([128, DS, 512], BF, tag="vt")
            ntok = BS * len(blocks)
            for m, pst in proj("wv", BS * blocks[0], ntok):
                nc.scalar.activation(
                    vt[:, m, :ntok],
                    pst[:, :ntok],
                    Ident,
                    bias=b_sb["bv"][:, m : m + 1],
                )
            return vt

        def vw_proj(vt, nblk):
            # (V @ wo)[tok, d_out] -> per-block [128, 2, D] (token subtiles)
            vws = [
                wvp.tile([128, 2, D], BF, tag="vw", name=f"vw{i}")
                for i in range(nblk)
            ]
            for ts in range(2 * nblk):
                for h in range(2):
                    pst = ps_pr.tile([128, 512], F32, tag="pr")
                    for kk in range(DS):
                        nc.tensor.matmul(
                            pst[:],
                            vt[:, kk, 128 * ts : 128 * (ts + 1)],
                            w_sb["wo"][:, kk, 512 * h : 512 * (h + 1)],
                            start=(kk == 0),
                            stop=(kk == DS - 1),
                        )
                    nc.vector.tensor_copy(
                        vws[ts // 2][:, ts % 2, 512 * h : 512 * (h + 1)], pst[:]
                    )
            return vws

        def _pair_q(k):
            # Queries of pair k: [local(k) | cross(k+1)]; k=0 is cross(1)
            # only, k=8 local(8) only.
            if k == 0:
                return 0, BS
            if k == 8:
                return BS * 7, BS
            return BS * (k - 1), 2 * BS

        def attend_scores(k, kt_k):
            q0, nq = _pair_q(k)
            pt = pp.tile([128, 2, 512], BF, tag="pt")
            for ks in range(2):
                pst = ps_sc.tile([128, 512], F32, tag="sc")
                for kk in range(DS):
                    nc.tensor.matmul(
                        pst[:, :nq],
                        kt_k[:, kk, 128 * ks : 128 * (ks + 1)],
                        qt[:, kk, q0 : q0 + nq],
                        start=(kk == 0),
                        stop=(kk == DS - 1),
                    )
                nc.scalar.activation(pt[:, ks, :nq], pst[:, :nq], Exp, scale=SCALE)
            bc = ps_bc.tile([128, 512], F32, tag="bc")
            for ks in range(2):
                nc.tensor.matmul(
                    bc[:, :nq],
                    ones_sb[:],
                    pt[:, ks, :nq],
                    start=(ks == 0),
                    stop=(ks == 1),
                )
            rc = rp.tile([128, 512], F32, tag="rc")
            nc.vector.reciprocal_approx_fast(rc[:, :nq], bc[:, :nq])
            return pt, rc

        def attend_out(k, pt, rc, vw_k):
            q0, nq = _pair_q(k)
            for m in range(DS):
                pso = ps_op.tile([128, 512], F32, tag="op")
                for ks in range(2):
                    nc.tensor.matmul(
                        pso[:, :nq],
                        vw_k[:, ks, 128 * m : 128 * (m + 1)],
                        pt[:, ks, :nq],
                        start=(ks == 0),
                        stop=(ks == 1),
                    )
                ost = op_sb.tile([128, 512], BF, tag="os")
                nc.vector.tensor_mul(ost[:, :nq], pso[:, :nq], rc[:, :nq])
                r = outt[128 * m : 128 * (m + 1), :]
                eng = nc.sync
                # Pair-major output layout (host reorders): pair k at cols
                # [512(k-1), 512k); the two half-attends at 3584 and 3840.
                if k == 0:
                    eng.dma_start(r[:, 14 * BS : 15 * BS], ost[:, :BS])
                elif k == 8:
                    eng.dma_start(r[:, 15 * BS : 16 * BS], ost[:, :BS])
                else:
                    c = 2 * BS * (k - 1)
                    eng.dma_start(r[:, c : c + 2 * BS], ost[:, : 2 * BS])

        def attend(k, kt_k, vw_k):
            pt, rc = attend_scores(k, kt_k)
            attend_out(k, pt, rc, vw_k)

        # Prologue. Per-queue DMA sustains only ~70-115 GB/s, so the 12.7 MB
        # of inputs is spread so each tensor lands just before its compute
        # phase: compute runs K -> Q -> scores(0) -> V -> VW -> out(0), with
        # weights arriving in that order (the scalar queue shares the ACT
        # sequencer, so it only carries early loads).
        sy, sc, gp = nc.sync, nc.scalar, nc.gpsimd
        load_x([0])
        load_w("wk", engs=(sy, sy, sy, sy, sy, sc, sc, sc))
        load_w("wq", engs=(sy, sy, sy, sy, gp, gp, gp, gp))
        load_w("wv", engs=(sy, sy, sy, sy, sc, sc, sc, sc))
        load_w("wo", engs=(gp, gp, gp, gp, sy, sy, sy, sy))
        load_x([1, 2, 3, 4])
        kts = {}
        vws = {}
        kts[0], kts[1] = proj_k((0, 1))
        proj_q(1, 2)
        pt0, rc0 = attend_scores(0, kts[0])
        vt = proj_v((0, 1))
        vws[0], vws[1] = vw_proj(vt, 2)
        attend_out(0, pt0, rc0, vws[0])
        for k in range(1, 7):
            if k % 2 == 1:  # KV group (k+1, k+2)
                kts[k + 1], kts[k + 2] = proj_k((k + 1, k + 2))
                vt = proj_v((k + 1, k + 2))
                vws[k + 1], vws[k + 2] = vw_proj(vt, 2)
            else:  # Q group (k+1, k+2)
                proj_q(k + 1, 2)
            attend(k, kts[k], vws[k])
        (kts[8],) = proj_k((8,))
        vt = proj_v((8,))
        (vws[8],) = vw_proj(vt, 1)
        attend(7, kts[7], vws[7])
        attend(8, kts[8], vws[8])

    nc.compile()
    return nc


def _get_nc():
    global _CACHED_NC
    if _CACHED_NC is None:
        _CACHED_NC = _build()
    return _CACHED_NC


def _make_in_maps(x, wq, bq, wk, bk, wv, bv, wo):
    base = {
        "wq": np.ascontiguousarray(wq.astype(BF_NP)),
        "wk": np.ascontiguousarray(wk.astype(BF_NP)),
        "wv": np.ascontiguousarray(wv.astype(BF_NP)),
        "wo": np.ascontiguousarray(wo.astype(BF_NP)),
        "bq": np.ascontiguousarray(bq.reshape(DS, 128).T, np.float32),
        "bk": np.ascontiguousarray(bk.reshape(DS, 128).T, np.float32),
        "bv": np.ascontiguousarray(bv.reshape(DS, 128).T, np.float32),
        "ones2d": np.ones((128, 128), BF_NP),
    }
    in_maps = []
    for c in range(8):
        b, t = c // 2, c % 2
        if t == 0:
            xkv = np.concatenate([x[b, 0:BS], x[b, 0 : 8 * BS]], axis=0)
        else:
            xkv = x[b, 8 * BS - BS : 16 * BS]
        in_maps.append(
            {**base, "xt": np.ascontiguousarray(xkv.T.astype(BF_NP))}
        )
    return in_maps


def _out_perm():
    # dst seg-row -> src row of the device's pair-major out^T layout.
    perm = np.empty(4096, np.intp)
    for k in range(1, 8):
        c = 512 * (k - 1)
        perm[c : c + 256] = np.arange(c, c + 256)  # local(k)
        perm[512 * k + 256 : 512 * k + 512] = np.arange(c + 256, c + 512)  # cross(k+1)
    perm[256:512] = np.arange(3584, 3840)  # cross(1)
    perm[3584:3840] = np.arange(3840, 4096)  # local(8)
    return perm


_PERM = _out_perm()


def _assemble(results, bo):
    out = np.empty((4, 16 * 2 * BS, D), np.float32)
    for c in range(8):
        b, t = c // 2, c % 2
        seg = 8 * 2 * BS  # 4096 output rows per core
        out[b, seg * t : seg * (t + 1), :] = (
            results[c]["outt"].T[_PERM].astype(np.float32)
        )
    out += np.asarray(bo, np.float32).reshape(1, 1, D)
    return out


def run(x, wq, bq, wk, bk, wv, bv, wo, bo, trace=False):
    nc = _get_nc()
    in_maps = _make_in_maps(x, wq, bq, wk, bk, wv, bv, wo)
    res = bass_utils.run_bass_kernel_spmd(
        nc, in_maps, core_ids=list(range(8)), trace=trace
    )
    return _assemble(res.results, bo), res


def kernel(x, wq, bq, wk, bk, wv, bv, wo, bo, block_size):
    assert int(block_size) == BS
    x = np.asarray(x, np.float32)
    assert x.shape == (4, 16 * BS, D), x.shape
    args = [np.asarray(a, np.float32) for a in (wq, bq, wk, bk, wv, bv, wo, bo)]
    wq, bq, wk, bk, wv, bv, wo, bo = args
    out, _ = run(x, wq, bq, wk, bk, wv, bv, wo, bo, trace=False)
    return out
